# revision 8
# baseline (speedup 1.0000x reference)
"""CRF negative log-likelihood loss on 8 Trainium2 NeuronCores.

Meet-in-the-middle forward algorithm: the log-partition of each sequence is
computed from BOTH ends simultaneously, halving the serial chain from T+1 to
ceil(T/2) steps. Per direction the recurrence is linear in the exponential
domain: alpha' = g ∘ (A_F alpha) forward, gamma' = g ∘ (A_B gamma) backward
(gamma_t = g_t ∘ beta_t), with A_F = exp(trans) and A_B = exp(trans)^T on the
48 real states. Both directions run in ONE chain per core via a block-diagonal
[98x98] stationary: partitions 0-48 forward block, 49-97 backward block, with
a 49th HOLD state per block whose injection row delay-starts each lane so all
segments end exactly at the final slot. Per step: one PE matmul (bf16) + one
DVE Hadamard (bf16 out) — the chain is latency-bound, so halving its length
halves device time. The meet logZ_b = log(alpha^T E^T gamma) happens on host,
as does the gold path score.

Data parallel over batch: 64 sequences per core, no collectives (the mean is
taken on host from the 8 cores' final states).
"""
import numpy as np
from contextlib import ExitStack

import concourse.bacc as bacc
import concourse.bass as bass
import concourse.tile as tile
from concourse import mybir
from concourse.bass_utils import run_bass_kernel_spmd

B, T, K = 512, 1024, 48
START, STOP = 46, 47
NEG = -10000.0
KA = 49          # 48 real states + hold (per direction)
H = 48
KB = 2 * KA      # 98: fwd block + bwd block
NCORES = 8
BC = B // NCORES  # 64 lanes per core
S = 512          # chain slots = ceil(T/2)
CH = 32          # steps per emission chunk

_nc_cache = {}


def _build_module(s=S, ch=CH, nsteps=None, nchains=2):
    """nsteps: number of recurrence steps to emit (default s); emissions DRAM
    is always [KB, s, BC] so timing variants with nsteps != s move identical
    bytes (nsteps > s wraps, correctness only for nsteps == s). nchains splits
    the BC lanes into independent column-sliced chains that pipeline on the
    engines."""
    if nsteps is None:
        nsteps = s
    key = ("nc", s, ch, nsteps, nchains)
    if key in _nc_cache:
        return _nc_cache[key]
    nc = bacc.Bacc(
        "TRN2",
        target_bir_lowering=False,
        debug=False,
        enable_asserts=False,
        num_devices=NCORES,
    )
    f32 = mybir.dt.float32
    bf16 = mybir.dt.bfloat16
    e_dram = nc.dram_tensor("etil", [KB, KB], bf16, kind="ExternalInput").ap()
    g_dram = nc.dram_tensor("emis", [KB, s, BC], f32, kind="ExternalInput").ap()
    w0_dram = nc.dram_tensor("w0", [KB, BC], bf16, kind="ExternalInput").ap()
    o_dram = nc.dram_tensor("wout", [KB, BC], f32, kind="ExternalOutput").ap()

    with tile.TileContext(nc) as tc:
        with ExitStack() as ctx:
            const = ctx.enter_context(tc.tile_pool(name="const", bufs=1))
            wpool = ctx.enter_context(tc.tile_pool(name="wp", bufs=4))
            graw_p = ctx.enter_context(tc.tile_pool(name="graw", bufs=2))
            gexp_p = ctx.enter_context(tc.tile_pool(name="gexp", bufs=2))
            psum_p = ctx.enter_context(
                tc.tile_pool(name="ps", bufs=max(2, 8 // nchains), space="PSUM")
            )

            etile = const.tile([KB, KB], bf16)
            nc.sync.dma_start(out=etile, in_=e_dram)

            cw = BC // nchains  # columns per chain
            w0t = const.tile([KB, BC], bf16)
            nc.sync.dma_start(out=w0t, in_=w0_dram)
            ws = [w0t[:, q * cw : (q + 1) * cw] for q in range(nchains)]

            wf = const.tile([KB, BC], f32)  # final fp32 copy for readout

            nstep = 0
            while nstep < nsteps:
                s0m = nstep % s  # wrap for timing variants with nsteps > s
                ns = min(ch, nsteps - nstep, s - s0m)
                graw = graw_p.tile([KB, ch, BC], f32, tag="graw")
                nc.sync.dma_start(
                    out=graw[:, :ns, :], in_=g_dram[:, s0m : s0m + ns, :]
                )
                gexp = gexp_p.tile([KB, ch, BC], f32, tag="gexp")
                nc.scalar.activation(
                    out=gexp[:, :ns, :],
                    in_=graw[:, :ns, :],
                    func=mybir.ActivationFunctionType.Exp,
                )
                for si in range(ns):
                    last = nstep + si == nsteps - 1
                    pss = []
                    for q in range(nchains):
                        ps = psum_p.tile([KB, cw], f32, tag=f"ps{q}")
                        nc.tensor.matmul(ps, etile, ws[q], start=True, stop=True)
                        pss.append(ps)
                    for q in range(nchains):
                        gsl = gexp[:, si, q * cw : (q + 1) * cw]
                        if last:
                            nc.vector.tensor_mul(
                                wf[:, q * cw : (q + 1) * cw], pss[q], gsl
                            )
                        else:
                            w2 = wpool.tile([KB, cw], bf16, tag=f"w{q}")
                            nc.vector.tensor_mul(w2, pss[q], gsl)
                            ws[q] = w2
                nstep += ns
            assert nstep == nsteps
            nc.sync.dma_start(out=o_dram, in_=wf)

    nc.compile()
    _nc_cache[key] = nc
    return nc


def _calibrate(feats, seq_len, trans):
    """Mean per-step log-growth (beyond per-step max emission) of the fwd and
    bwd recurrences, measured in fp64 on a few sequences, batched."""
    E64 = np.exp(trans.astype(np.float64))  # E[n,p]
    nb, nt = 4, 192
    f = feats[:nb, :nt].astype(np.float64)  # [nb,nt,K]
    mx = f.max(axis=2)                      # [nb,nt]
    fv = np.full((nb, K), NEG)
    fv[:, START] = 0.0
    dF = np.empty((nb, nt))
    for t in range(nt):
        m = fv.max(axis=1)
        wv = np.exp(fv - m[:, None])
        fv = np.log(wv @ E64.T) + m[:, None] + f[:, t]
        dF[:, t] = fv.max(axis=1) - m - mx[:, t]
    bv = np.broadcast_to(trans[STOP, :K].astype(np.float64), (nb, K)).copy()
    dB = np.empty((nb, nt))
    for j in range(nt):
        t = nt - 1 - j
        m = bv.max(axis=1)
        wv = np.exp(bv - m[:, None])
        bv = np.log(wv @ E64) + m[:, None] + f[:, t]
        dB[:, j] = bv.max(axis=1) - m - mx[:, t]
    return float(dF.mean()), float(dB.mean())


def _host_prep(feats, seq_len, trans):
    """Build per-core emission tensors [KB, S, BC], the block-diag stationary,
    initial state, and the shift totals needed for readout."""
    feats = np.ascontiguousarray(feats, dtype=np.float32)
    seq_len = np.asarray(seq_len, dtype=np.int64)
    trans = np.asarray(trans, dtype=np.float32)

    E = np.exp(trans)                   # [K,K] E[n,p]; NEG rows/cols -> 0
    muF, muB = _calibrate(feats, seq_len, trans)

    F = (seq_len + 1) // 2              # fwd steps per sequence, >= 1
    R = seq_len - F                     # bwd steps, >= 0

    mx = feats.max(axis=2)              # [B,T]
    cF = mx + muF                       # shift applied at fwd step t (0-based)
    cB = mx + muB

    sidx = np.arange(S)[None, :]        # [1,S]
    bidx = np.arange(B)[:, None]

    # fwd block: lane b live in slots [S-F_b, S), internal index i = s-(S-F_b)
    iF = sidx - (S - F[:, None])        # [B,S]
    liveF = iF >= 0
    iFc = np.clip(iF, 0, T - 1)
    emF = np.empty((B, S, KA), dtype=np.float32)
    emF[:, :, :K] = np.where(
        liveF[:, :, None],
        feats[bidx, iFc] - cF[bidx, iFc][:, :, None],
        NEG,
    )
    emF[:, :, H] = np.where(liveF, NEG, 0.0)

    # bwd block: live in slots [S-R_b, S), consumes t = L-1-j (0-based)
    jB = sidx - (S - R[:, None])
    liveB = jB >= 0
    tB = np.clip(seq_len[:, None] - 1 - jB, 0, T - 1)
    emB = np.empty((B, S, KA), dtype=np.float32)
    emB[:, :, :K] = np.where(
        liveB[:, :, None],
        feats[bidx, tB] - cB[bidx, tB][:, :, None],
        NEG,
    )
    emB[:, :, H] = np.where(liveB, NEG, 0.0)

    # shift totals per sequence
    m64 = mx.astype(np.float64)
    mcum = np.concatenate(
        [np.zeros((B, 1)), np.cumsum(m64, axis=1)], axis=1
    )  # [B,T+1]
    Cf = mcum[np.arange(B), F] + muF * F                       # sum mx[0:F]
    Cb = (mcum[np.arange(B), seq_len] - mcum[np.arange(B), F]) + muB * R

    # stationary: lhsT[p,n] = A[n,p]; block-diag(fwd, bwd)
    lf = np.zeros((KA, KA), dtype=np.float32)
    lf[:K, :K] = E.T                 # A_F real = E
    lf[H, :K] = E[:, START]          # injection row: HOLD -> start
    lf[H, H] = 1.0
    lb = np.zeros((KA, KA), dtype=np.float32)
    lb[:K, :K] = E                   # A_B real = E^T
    lb[H, :K] = E[STOP, :K]          # injection row: HOLD -> beta_L
    lb[H, H] = 1.0
    import ml_dtypes
    etil = np.zeros((KB, KB), dtype=np.float32)
    etil[:KA, :KA] = lf
    etil[KA:, KA:] = lb
    etil = etil.astype(ml_dtypes.bfloat16)

    w0 = np.zeros((KB, BC), dtype=ml_dtypes.bfloat16)
    w0[H, :] = 1.0
    w0[KA + H, :] = 1.0

    per_core = []
    for cix in range(NCORES):
        sl = slice(cix * BC, (cix + 1) * BC)
        blk = np.concatenate([emF[sl], emB[sl]], axis=2)  # [BC, S, 98]
        per_core.append(np.ascontiguousarray(blk.transpose(2, 1, 0)))

    return per_core, etil, w0, (F, R, Cf, Cb, E)


def _gold_score(feats, tags, seq_len, trans):
    feats = np.asarray(feats, dtype=np.float32)
    tags = np.asarray(tags, dtype=np.int64)
    seq_len = np.asarray(seq_len, dtype=np.int64)
    trans = np.asarray(trans, dtype=np.float32)
    tags_ext = np.concatenate(
        [np.full((B, 1), START, dtype=np.int64), tags], axis=1
    )
    trans_sc = trans[tags_ext[:, 1:], tags_ext[:, :-1]]
    emit_sc = np.take_along_axis(feats, tags_ext[:, 1:, None], axis=2)[..., 0]
    mask = np.arange(T)[None, :] < seq_len[:, None]
    last_tag = np.take_along_axis(tags_ext, seq_len[:, None], axis=1)[:, 0]
    gold = (
        np.where(mask, trans_sc + emit_sc, 0.0).sum(1, dtype=np.float64)
        + trans[STOP, last_tag]
    )
    return gold  # [B] f64


def kernel(feats, tags, seq_len, transitions):
    feats = np.asarray(feats)
    per_core, etil, w0, (F, R, Cf, Cb, E) = _host_prep(
        feats, seq_len, transitions
    )
    nc = _build_module()
    in_maps = [
        {"etil": etil, "emis": per_core[c], "w0": w0} for c in range(NCORES)
    ]
    res = run_bass_kernel_spmd(nc, in_maps, list(range(NCORES)))
    wout = np.concatenate(
        [np.asarray(res.results[c]["wout"]) for c in range(NCORES)], axis=1
    ).astype(np.float64)  # [KB, B]

    alpha = wout[:K, :].T           # [B, K]
    gamma = wout[KA : KA + K, :].T  # [B, K]
    E64 = np.asarray(E, dtype=np.float64)
    beta_F = gamma @ E64            # beta_F[b,p] = sum_n gamma[b,n] E[n,p]
    bL = np.exp(np.asarray(transitions, np.float64)[STOP, :K])
    dot = np.where(
        R > 0,
        np.einsum("bp,bp->b", alpha, beta_F),
        alpha @ bL,
    )
    forward_score = np.log(dot) + Cf + Cb
    gold = _gold_score(feats, tags, seq_len, transitions)
    loss = np.mean(forward_score - gold)
    return np.float32(loss)


# revision 11
# speedup vs baseline: 1.6676x; 1.6676x over previous
"""CRF negative log-likelihood loss on 8 Trainium2 NeuronCores.

Meet-in-the-middle forward algorithm: the log-partition of each sequence is
computed from BOTH ends simultaneously, halving the serial chain from T+1 to
ceil(T/2) steps. Per direction the recurrence is linear in the exponential
domain: alpha' = g ∘ (A_F alpha) forward, gamma' = g ∘ (A_B gamma) backward
(gamma_t = g_t ∘ beta_t), with A_F = exp(trans) and A_B = exp(trans)^T on the
48 real states. Both directions run in ONE chain per core via a block-diagonal
[98x98] stationary: partitions 0-48 forward block, 49-97 backward block, with
a 49th HOLD state per block whose injection row delay-starts each lane so all
segments end exactly at the final slot. Per step: one PE matmul (bf16) + one
DVE Hadamard (bf16 out) — the chain is latency-bound, so halving its length
halves device time. The meet logZ_b = log(alpha^T E^T gamma) happens on host,
as does the gold path score.

Data parallel over batch: 64 sequences per core, no collectives (the mean is
taken on host from the 8 cores' final states).
"""
import numpy as np
from contextlib import ExitStack

import concourse.bacc as bacc
import concourse.bass as bass
import concourse.tile as tile
from concourse import mybir
from concourse.bass_utils import run_bass_kernel_spmd

B, T, K = 512, 1024, 48
START, STOP = 46, 47
NEG = -10000.0
KA = 49          # 48 real states + hold (per direction)
H = 48
KB = 2 * KA      # 98: fwd block + bwd block
NCORES = 8
BC = B // NCORES  # 64 lanes per core
S = 512          # chain slots = ceil(T/2)
CH = 32          # steps per emission chunk

_nc_cache = {}


def _build_module(s=S, ch=CH, nsteps=None, nchains=2):
    """nsteps: number of recurrence steps to emit (default s); emissions DRAM
    is always [KB, s, BC] so timing variants with nsteps != s move identical
    bytes (nsteps > s wraps, correctness only for nsteps == s). nchains splits
    the BC lanes into independent column-sliced chains that pipeline on the
    engines."""
    if nsteps is None:
        nsteps = s
    key = ("nc", s, ch, nsteps, nchains)
    if key in _nc_cache:
        return _nc_cache[key]
    nc = bacc.Bacc(
        "TRN2",
        target_bir_lowering=False,
        debug=False,
        enable_asserts=False,
        num_devices=NCORES,
    )
    f32 = mybir.dt.float32
    bf16 = mybir.dt.bfloat16
    e_dram = nc.dram_tensor("etil", [KB, KB], bf16, kind="ExternalInput").ap()
    g_dram = nc.dram_tensor("emis", [KB, s, BC], f32, kind="ExternalInput").ap()
    w0_dram = nc.dram_tensor("w0", [KB, BC], bf16, kind="ExternalInput").ap()
    o_dram = nc.dram_tensor("wout", [KB, BC], f32, kind="ExternalOutput").ap()

    with tile.TileContext(nc) as tc:
        with ExitStack() as ctx:
            const = ctx.enter_context(tc.tile_pool(name="const", bufs=1))
            wpool = ctx.enter_context(tc.tile_pool(name="wp", bufs=4))
            graw_p = ctx.enter_context(tc.tile_pool(name="graw", bufs=2))
            gexp_p = ctx.enter_context(tc.tile_pool(name="gexp", bufs=2))
            psum_p = ctx.enter_context(
                tc.tile_pool(name="ps", bufs=max(2, 8 // nchains), space="PSUM")
            )

            etile = const.tile([KB, KB], bf16)
            nc.sync.dma_start(out=etile, in_=e_dram)

            cw = BC // nchains  # columns per chain
            w0t = const.tile([KB, BC], bf16)
            nc.sync.dma_start(out=w0t, in_=w0_dram)
            ws = [w0t[:, q * cw : (q + 1) * cw] for q in range(nchains)]

            wf = const.tile([KB, BC], f32)  # final fp32 copy for readout

            nstep = 0
            while nstep < nsteps:
                s0m = nstep % s  # wrap for timing variants with nsteps > s
                ns = min(ch, nsteps - nstep, s - s0m)
                graw = graw_p.tile([KB, ch, BC], f32, tag="graw")
                nc.sync.dma_start(
                    out=graw[:, :ns, :], in_=g_dram[:, s0m : s0m + ns, :]
                )
                gexp = gexp_p.tile([KB, ch, BC], f32, tag="gexp")
                nc.scalar.activation(
                    out=gexp[:, :ns, :],
                    in_=graw[:, :ns, :],
                    func=mybir.ActivationFunctionType.Exp,
                )
                for si in range(ns):
                    last = nstep + si == nsteps - 1
                    pss = []
                    for q in range(nchains):
                        ps = psum_p.tile([KB, cw], f32, tag=f"ps{q}")
                        nc.tensor.matmul(ps, etile, ws[q], start=True, stop=True)
                        pss.append(ps)
                    for q in range(nchains):
                        gsl = gexp[:, si, q * cw : (q + 1) * cw]
                        if last:
                            nc.vector.tensor_mul(
                                wf[:, q * cw : (q + 1) * cw], pss[q], gsl
                            )
                        else:
                            w2 = wpool.tile([KB, cw], bf16, tag=f"w{q}")
                            nc.vector.tensor_mul(w2, pss[q], gsl)
                            ws[q] = w2
                nstep += ns
            assert nstep == nsteps
            nc.sync.dma_start(out=o_dram, in_=wf)

    nc.compile()
    _nc_cache[key] = nc
    return nc


def _calibrate(feats, seq_len, trans):
    """Mean per-step log-growth (beyond per-step max emission) of the fwd and
    bwd recurrences, measured in fp64 on a few sequences, batched."""
    E64 = np.exp(trans.astype(np.float64))  # E[n,p]
    nb, nt = 4, 192
    f = feats[:nb, :nt].astype(np.float64)  # [nb,nt,K]
    mx = f.max(axis=2)                      # [nb,nt]
    fv = np.full((nb, K), NEG)
    fv[:, START] = 0.0
    dF = np.empty((nb, nt))
    for t in range(nt):
        m = fv.max(axis=1)
        wv = np.exp(fv - m[:, None])
        fv = np.log(wv @ E64.T) + m[:, None] + f[:, t]
        dF[:, t] = fv.max(axis=1) - m - mx[:, t]
    bv = np.broadcast_to(trans[STOP, :K].astype(np.float64), (nb, K)).copy()
    dB = np.empty((nb, nt))
    for j in range(nt):
        t = nt - 1 - j
        m = bv.max(axis=1)
        wv = np.exp(bv - m[:, None])
        bv = np.log(wv @ E64) + m[:, None] + f[:, t]
        dB[:, j] = bv.max(axis=1) - m - mx[:, t]
    return float(dF.mean()), float(dB.mean())


def _host_prep(feats, seq_len, trans):
    """Build per-core emission tensors [KB, S, BC], the block-diag stationary,
    initial state, and the shift totals needed for readout."""
    feats = np.ascontiguousarray(feats, dtype=np.float32)
    seq_len = np.asarray(seq_len, dtype=np.int64)
    trans = np.asarray(trans, dtype=np.float32)

    E = np.exp(trans)                   # [K,K] E[n,p]; NEG rows/cols -> 0
    muF, muB = _calibrate(feats, seq_len, trans)

    F = (seq_len + 1) // 2              # fwd steps per sequence, >= 1
    R = seq_len - F                     # bwd steps, >= 0

    mx = feats.max(axis=2)              # [B,T]
    cF = mx + muF                       # shift applied at fwd step t (0-based)
    cB = mx + muB

    sidx = np.arange(S)[None, :]        # [1,S]

    em_all = np.full((B, S, KB), NEG, dtype=np.float32)

    # fwd block: lane b live in slots [S-F_b, S), internal index i = s-(S-F_b)
    iF = sidx - (S - F[:, None])        # [B,S]
    liveF = iF >= 0
    bl, sl_ = np.nonzero(liveF)
    tl = iF[bl, sl_]
    em_all[bl, sl_, :K] = feats[bl, tl] - cF[bl, tl][:, None]
    em_all[:, :, H] = np.where(liveF, NEG, 0.0)

    # bwd block: live in slots [S-R_b, S), consumes t = L-1-j (0-based)
    jB = sidx - (S - R[:, None])
    liveB = jB >= 0
    bl, sl_ = np.nonzero(liveB)
    tl = seq_len[bl] - 1 - jB[bl, sl_]
    em_all[bl, sl_, KA : KA + K] = feats[bl, tl] - cB[bl, tl][:, None]
    em_all[:, :, KA + H] = np.where(liveB, NEG, 0.0)

    # shift totals per sequence
    m64 = mx.astype(np.float64)
    mcum = np.concatenate(
        [np.zeros((B, 1)), np.cumsum(m64, axis=1)], axis=1
    )  # [B,T+1]
    Cf = mcum[np.arange(B), F] + muF * F                       # sum mx[0:F]
    Cb = (mcum[np.arange(B), seq_len] - mcum[np.arange(B), F]) + muB * R

    # stationary: lhsT[p,n] = A[n,p]; block-diag(fwd, bwd)
    lf = np.zeros((KA, KA), dtype=np.float32)
    lf[:K, :K] = E.T                 # A_F real = E
    lf[H, :K] = E[:, START]          # injection row: HOLD -> start
    lf[H, H] = 1.0
    lb = np.zeros((KA, KA), dtype=np.float32)
    lb[:K, :K] = E                   # A_B real = E^T
    lb[H, :K] = E[STOP, :K]          # injection row: HOLD -> beta_L
    lb[H, H] = 1.0
    import ml_dtypes
    etil = np.zeros((KB, KB), dtype=np.float32)
    etil[:KA, :KA] = lf
    etil[KA:, KA:] = lb
    etil = etil.astype(ml_dtypes.bfloat16)

    w0 = np.zeros((KB, BC), dtype=ml_dtypes.bfloat16)
    w0[H, :] = 1.0
    w0[KA + H, :] = 1.0

    per_core = [
        np.ascontiguousarray(
            em_all[cix * BC : (cix + 1) * BC].transpose(2, 1, 0)
        )
        for cix in range(NCORES)
    ]

    return per_core, etil, w0, (F, R, Cf, Cb, E)


def _gold_score(feats, tags, seq_len, trans):
    feats = np.asarray(feats, dtype=np.float32)
    tags = np.asarray(tags, dtype=np.int64)
    seq_len = np.asarray(seq_len, dtype=np.int64)
    trans = np.asarray(trans, dtype=np.float32)
    tags_ext = np.concatenate(
        [np.full((B, 1), START, dtype=np.int64), tags], axis=1
    )
    trans_sc = trans[tags_ext[:, 1:], tags_ext[:, :-1]]
    emit_sc = np.take_along_axis(feats, tags_ext[:, 1:, None], axis=2)[..., 0]
    mask = np.arange(T)[None, :] < seq_len[:, None]
    last_tag = np.take_along_axis(tags_ext, seq_len[:, None], axis=1)[:, 0]
    gold = (
        np.where(mask, trans_sc + emit_sc, 0.0).sum(1, dtype=np.float64)
        + trans[STOP, last_tag]
    )
    return gold  # [B] f64


def kernel(feats, tags, seq_len, transitions):
    feats = np.asarray(feats)
    per_core, etil, w0, (F, R, Cf, Cb, E) = _host_prep(
        feats, seq_len, transitions
    )
    nc = _build_module()
    in_maps = [
        {"etil": etil, "emis": per_core[c], "w0": w0} for c in range(NCORES)
    ]
    try:
        res = run_bass_kernel_spmd(nc, in_maps, list(range(NCORES)))
    except Exception:
        # transient NRT execution faults have been observed once after large
        # prior workloads; a clean re-dispatch recovers
        res = run_bass_kernel_spmd(nc, in_maps, list(range(NCORES)))
    wout = np.concatenate(
        [np.asarray(res.results[c]["wout"]) for c in range(NCORES)], axis=1
    ).astype(np.float64)  # [KB, B]

    alpha = wout[:K, :].T           # [B, K]
    gamma = wout[KA : KA + K, :].T  # [B, K]
    E64 = np.asarray(E, dtype=np.float64)
    beta_F = gamma @ E64            # beta_F[b,p] = sum_n gamma[b,n] E[n,p]
    bL = np.exp(np.asarray(transitions, np.float64)[STOP, :K])
    dot = np.where(
        R > 0,
        np.einsum("bp,bp->b", alpha, beta_F),
        alpha @ bL,
    )
    forward_score = np.log(dot) + Cf + Cb
    gold = _gold_score(feats, tags, seq_len, transitions)
    loss = np.mean(forward_score - gold)
    return np.float32(loss)


# revision 13
# speedup vs baseline: 2.4904x; 1.4934x over previous
"""CRF negative log-likelihood loss on 8 Trainium2 NeuronCores.

Meet-in-the-middle forward algorithm: the log-partition of each sequence is
computed from BOTH ends simultaneously, halving the serial chain from T+1 to
ceil(T/2) steps. Per direction the recurrence is linear in the exponential
domain: alpha' = g ∘ (A_F alpha) forward, gamma' = g ∘ (A_B gamma) backward
(gamma_t = g_t ∘ beta_t), with A_F = exp(trans) and A_B = exp(trans)^T on the
48 real states. Both directions run in ONE chain per core via a block-diagonal
[98x98] stationary: partitions 0-48 forward block, 49-97 backward block, with
a 49th HOLD state per block whose injection row delay-starts each lane so all
segments end exactly at the final slot. Per step: one PE matmul (bf16) + one
DVE Hadamard (bf16 out) — the chain is latency-bound, so halving its length
halves device time. The meet logZ_b = log(alpha^T E^T gamma) happens on host,
as does the gold path score.

Data parallel over batch: 64 sequences per core, no collectives (the mean is
taken on host from the 8 cores' final states).
"""
import numpy as np
from contextlib import ExitStack

import concourse.bacc as bacc
import concourse.tile as tile
from concourse import mybir
from concourse.bass_utils import run_bass_kernel_spmd

B, T, K = 512, 1024, 48
START, STOP = 46, 47
NEG = -10000.0
KA = 49          # 48 real states + hold (per direction)
H = 48
KB = 2 * KA      # 98: fwd block + bwd block
NCORES = 8
BC = B // NCORES  # 64 lanes per core
S = 512          # chain slots = ceil(T/2)
CH = 32          # steps per emission chunk

_nc_cache = {}


def _build_module(s=S, ch=CH, nsteps=None, nchains=2):
    """nsteps: number of recurrence steps to emit (default s); emissions DRAM
    is always [KB, s, BC] so timing variants with nsteps != s move identical
    bytes (nsteps > s wraps, correctness only for nsteps == s). nchains splits
    the BC lanes into independent column-sliced chains that pipeline on the
    engines."""
    if nsteps is None:
        nsteps = s
    key = ("nc", s, ch, nsteps, nchains)
    if key in _nc_cache:
        return _nc_cache[key]
    nc = bacc.Bacc(
        "TRN2",
        target_bir_lowering=False,
        debug=False,
        enable_asserts=False,
        num_devices=NCORES,
    )
    f32 = mybir.dt.float32
    bf16 = mybir.dt.bfloat16
    e_dram = nc.dram_tensor("etil", [KB, KB], bf16, kind="ExternalInput").ap()
    g_dram = nc.dram_tensor("emis", [KB, s, BC], f32, kind="ExternalInput").ap()
    w0_dram = nc.dram_tensor("w0", [KB, BC], bf16, kind="ExternalInput").ap()
    o_dram = nc.dram_tensor("wout", [KB, BC], f32, kind="ExternalOutput").ap()

    with tile.TileContext(nc) as tc:
        with ExitStack() as ctx:
            const = ctx.enter_context(tc.tile_pool(name="const", bufs=1))
            wpool = ctx.enter_context(tc.tile_pool(name="wp", bufs=4))
            graw_p = ctx.enter_context(tc.tile_pool(name="graw", bufs=2))
            gexp_p = ctx.enter_context(tc.tile_pool(name="gexp", bufs=2))
            psum_p = ctx.enter_context(
                tc.tile_pool(name="ps", bufs=max(2, 8 // nchains), space="PSUM")
            )

            etile = const.tile([KB, KB], bf16)
            nc.sync.dma_start(out=etile, in_=e_dram)

            cw = BC // nchains  # columns per chain
            w0t = const.tile([KB, BC], bf16)
            nc.sync.dma_start(out=w0t, in_=w0_dram)
            ws = [w0t[:, q * cw : (q + 1) * cw] for q in range(nchains)]

            wf = const.tile([KB, BC], f32)  # final fp32 copy for readout

            nstep = 0
            while nstep < nsteps:
                s0m = nstep % s  # wrap for timing variants with nsteps > s
                ns = min(ch, nsteps - nstep, s - s0m)
                graw = graw_p.tile([KB, ch, BC], f32, tag="graw")
                nc.sync.dma_start(
                    out=graw[:, :ns, :], in_=g_dram[:, s0m : s0m + ns, :]
                )
                gexp = gexp_p.tile([KB, ch, BC], f32, tag="gexp")
                nc.scalar.activation(
                    out=gexp[:, :ns, :],
                    in_=graw[:, :ns, :],
                    func=mybir.ActivationFunctionType.Exp,
                )
                for si in range(ns):
                    last = nstep + si == nsteps - 1
                    pss = []
                    for q in range(nchains):
                        ps = psum_p.tile([KB, cw], f32, tag=f"ps{q}")
                        nc.tensor.matmul(ps, etile, ws[q], start=True, stop=True)
                        pss.append(ps)
                    for q in range(nchains):
                        gsl = gexp[:, si, q * cw : (q + 1) * cw]
                        if last:
                            nc.vector.tensor_mul(
                                wf[:, q * cw : (q + 1) * cw], pss[q], gsl
                            )
                        else:
                            w2 = wpool.tile([KB, cw], bf16, tag=f"w{q}")
                            nc.vector.tensor_mul(w2, pss[q], gsl)
                            ws[q] = w2
                nstep += ns
            assert nstep == nsteps
            nc.sync.dma_start(out=o_dram, in_=wf)

    nc.compile()
    _nc_cache[key] = nc
    return nc


def _calibrate(feats, seq_len, trans):
    """Mean per-step log-growth (beyond per-step max emission) of the fwd and
    bwd recurrences, measured in fp64 on a few sequences, batched."""
    E64 = np.exp(trans.astype(np.float64))  # E[n,p]
    nb, nt = 4, 192
    f = feats[:nb, :nt].astype(np.float64)  # [nb,nt,K]
    mx = f.max(axis=2)                      # [nb,nt]
    fv = np.full((nb, K), NEG)
    fv[:, START] = 0.0
    dF = np.empty((nb, nt))
    for t in range(nt):
        m = fv.max(axis=1)
        wv = np.exp(fv - m[:, None])
        fv = np.log(wv @ E64.T) + m[:, None] + f[:, t]
        dF[:, t] = fv.max(axis=1) - m - mx[:, t]
    bv = np.broadcast_to(trans[STOP, :K].astype(np.float64), (nb, K)).copy()
    dB = np.empty((nb, nt))
    for j in range(nt):
        t = nt - 1 - j
        m = bv.max(axis=1)
        wv = np.exp(bv - m[:, None])
        bv = np.log(wv @ E64) + m[:, None] + f[:, t]
        dB[:, j] = bv.max(axis=1) - m - mx[:, t]
    return float(dF.mean()), float(dB.mean())


def _host_prep(feats, seq_len, trans):
    """Build per-core emission tensors [KB, S, BC], the block-diag stationary,
    initial state, and the shift totals needed for readout."""
    feats = np.ascontiguousarray(feats, dtype=np.float32)
    seq_len = np.asarray(seq_len, dtype=np.int64)
    trans = np.asarray(trans, dtype=np.float32)

    E = np.exp(trans)                   # [K,K] E[n,p]; NEG rows/cols -> 0
    muF, muB = _calibrate(feats, seq_len, trans)

    F = (seq_len + 1) // 2              # fwd steps per sequence, >= 1
    R = seq_len - F                     # bwd steps, >= 0

    mx = feats.max(axis=2)              # [B,T]
    cF = mx + muF                       # shift applied at fwd step t (0-based)
    cB = mx + muB

    sidx = np.arange(S)[None, :]        # [1,S]

    em_all = np.full((B, S, KB), NEG, dtype=np.float32)

    # fwd block: lane b live in slots [S-F_b, S), internal index i = s-(S-F_b)
    iF = sidx - (S - F[:, None])        # [B,S]
    liveF = iF >= 0
    bl, sl_ = np.nonzero(liveF)
    tl = iF[bl, sl_]
    em_all[bl, sl_, :K] = feats[bl, tl] - cF[bl, tl][:, None]
    em_all[:, :, H] = np.where(liveF, NEG, 0.0)

    # bwd block: live in slots [S-R_b, S), consumes t = L-1-j (0-based)
    jB = sidx - (S - R[:, None])
    liveB = jB >= 0
    bl, sl_ = np.nonzero(liveB)
    tl = seq_len[bl] - 1 - jB[bl, sl_]
    em_all[bl, sl_, KA : KA + K] = feats[bl, tl] - cB[bl, tl][:, None]
    em_all[:, :, KA + H] = np.where(liveB, NEG, 0.0)

    # shift totals per sequence
    m64 = mx.astype(np.float64)
    mcum = np.concatenate(
        [np.zeros((B, 1)), np.cumsum(m64, axis=1)], axis=1
    )  # [B,T+1]
    Cf = mcum[np.arange(B), F] + muF * F                       # sum mx[0:F]
    Cb = (mcum[np.arange(B), seq_len] - mcum[np.arange(B), F]) + muB * R

    # stationary: lhsT[p,n] = A[n,p]; block-diag(fwd, bwd)
    lf = np.zeros((KA, KA), dtype=np.float32)
    lf[:K, :K] = E.T                 # A_F real = E
    lf[H, :K] = E[:, START]          # injection row: HOLD -> start
    lf[H, H] = 1.0
    lb = np.zeros((KA, KA), dtype=np.float32)
    lb[:K, :K] = E                   # A_B real = E^T
    lb[H, :K] = E[STOP, :K]          # injection row: HOLD -> beta_L
    lb[H, H] = 1.0
    import ml_dtypes
    etil = np.zeros((KB, KB), dtype=np.float32)
    etil[:KA, :KA] = lf
    etil[KA:, KA:] = lb
    etil = etil.astype(ml_dtypes.bfloat16)

    w0 = np.zeros((KB, BC), dtype=ml_dtypes.bfloat16)
    w0[H, :] = 1.0
    w0[KA + H, :] = 1.0

    per_core = [
        np.ascontiguousarray(
            em_all[cix * BC : (cix + 1) * BC].transpose(2, 1, 0)
        )
        for cix in range(NCORES)
    ]

    return per_core, etil, w0, (F, R, Cf, Cb, E)


def _gold_score(feats, tags, seq_len, trans):
    feats = np.asarray(feats, dtype=np.float32)
    tags = np.asarray(tags, dtype=np.int64)
    seq_len = np.asarray(seq_len, dtype=np.int64)
    trans = np.asarray(trans, dtype=np.float32)
    tags_ext = np.concatenate(
        [np.full((B, 1), START, dtype=np.int64), tags], axis=1
    )
    trans_sc = trans[tags_ext[:, 1:], tags_ext[:, :-1]]
    emit_sc = np.take_along_axis(feats, tags_ext[:, 1:, None], axis=2)[..., 0]
    mask = np.arange(T)[None, :] < seq_len[:, None]
    last_tag = np.take_along_axis(tags_ext, seq_len[:, None], axis=1)[:, 0]
    gold = (
        np.where(mask, trans_sc + emit_sc, 0.0).sum(1, dtype=np.float64)
        + trans[STOP, last_tag]
    )
    return gold  # [B] f64


def kernel(feats, tags, seq_len, transitions):
    feats = np.asarray(feats)
    per_core, etil, w0, (F, R, Cf, Cb, E) = _host_prep(
        feats, seq_len, transitions
    )
    nc = _build_module()
    in_maps = [
        {"etil": etil, "emis": per_core[c], "w0": w0} for c in range(NCORES)
    ]
    E64 = np.asarray(E, dtype=np.float64)
    bL = np.exp(np.asarray(transitions, np.float64)[STOP, :K])
    gold = _gold_score(feats, tags, seq_len, transitions)

    # transient NRT faults (crashes or silently corrupted outputs) have been
    # observed rarely; validate the result and re-dispatch if it is not finite
    loss = None
    for attempt in range(3):
        try:
            res = run_bass_kernel_spmd(nc, in_maps, list(range(NCORES)))
        except Exception:
            if attempt == 2:
                raise
            continue
        wout = np.concatenate(
            [np.asarray(res.results[c]["wout"]) for c in range(NCORES)], axis=1
        ).astype(np.float64)  # [KB, B]
        alpha = wout[:K, :].T           # [B, K]
        gamma = wout[KA : KA + K, :].T  # [B, K]
        beta_F = gamma @ E64            # beta_F[b,p] = sum_n gamma[b,n] E[n,p]
        dot = np.where(
            R > 0,
            np.einsum("bp,bp->b", alpha, beta_F),
            alpha @ bL,
        )
        if not (np.isfinite(wout).all() and (dot > 0.0).all()):
            continue
        forward_score = np.log(dot) + Cf + Cb
        cand = np.mean(forward_score - gold)
        if np.isfinite(cand):
            loss = cand
            break
    if loss is None:
        raise RuntimeError("device kernel returned non-finite results")
    return np.float32(loss)
